# revision 2
# baseline (speedup 1.0000x reference)
"""Causal single-head attention (S=8192, dk=64) on 8 TRN2 NeuronCores.

Sharding: zigzag sequence-parallel over query rows. The 8192 rows form 16
blocks of 512; core b owns row-blocks {b, 15-b} so every core does exactly
17 block-sized (512 rows x 512 keys) units of causal work -> perfect load
balance, no collectives.

SPMD constraint (all cores share one instruction graph) is satisfied by
host-side packing: the host packs, per core, 17 "slots" of
(qT, kT, v_aug) operand tiles; slots 0 and 1 are the two diagonal
(triangular-masked) blocks for every core, the remaining 15 are full
blocks. The device graph is identical across cores; only data differs.

Device pipeline per slot (Tile framework handles sync):
  QK^T: 4 matmuls [K=64, M=128 keys, N=512 rows] -> sT in PSUM (f32).
        Slots are processed in two groups mapped to PE row-halves
        (tile_position (0,0) / (64,0)) so pairs of K=64 matmuls can run
        concurrently in the 128-row array.
  exp:  one ACT pass per [128,1024] PSUM tile: bf16 out = exp(s/64),
        fused scale, PSUM -> SBUF.
  mask: slots 0/1 only: gpsimd affine_select zeroes key>row entries.
  AV:   4 matmuls lhsT=v_aug[128 keys, 65] rhs=exp-tile -> PSUM [65,512]
        accumulated over key subtiles; row 64 of v_aug is ones -> row 64
        of the output is the softmax denominator.
  out:  DVE copy PSUM->SBUF, DMA per-slot partial [65,512] to HBM.

Host combines: per row-chunk, sum slot partials, divide by denominator row.
"""

import numpy as np
import ml_dtypes

S = 8192
DK = 64
BLK = 512  # row/key block
NB = S // BLK  # 16
N_CORES = 8
NSLOT = 17  # (b+1) + (16-b) block units per core
G0 = 9  # slots 0..8 -> PE rows 0:64, slots 9..16 -> PE rows 64:128
KSUB = 128  # key subtile (psum partition dim)
NKT = BLK // KSUB  # 4
QP_COLS = G0 * BLK  # 4608
VP_COLS = NSLOT * NKT * 65  # 4420

_BF16 = ml_dtypes.bfloat16
_CACHE = {}


def _core_slots(b):
    """Slot table for core b: list of (rowblock, keyblock, is_diag)."""
    A, B = b, 15 - b
    slots = [(A, A, True), (B, B, True)]
    slots += [(A, c, False) for c in range(A)]
    slots += [(B, c, False) for c in range(B)]
    assert len(slots) == NSLOT
    return slots


def _build_graph():
    import concourse.mybir as mybir
    import concourse.tile as tile
    from concourse import bacc

    f32 = mybir.dt.float32
    bf16 = mybir.dt.bfloat16

    nc = bacc.Bacc("TRN2", target_bir_lowering=False)
    qp = nc.declare_dram_parameter("qp", [128, QP_COLS], bf16, isOutput=False)
    kp = nc.declare_dram_parameter("kp", [128, QP_COLS], bf16, isOutput=False)
    vp = nc.declare_dram_parameter("vp", [128, VP_COLS], bf16, isOutput=False)
    op = nc.declare_dram_parameter("op", [NSLOT, 65, BLK], f32, isOutput=True)

    with tile.TileContext(nc) as tc:
        with (
            tc.tile_pool(name="data", bufs=1) as data,
            tc.tile_pool(name="stp", bufs=3, space="PSUM") as stp,
            tc.tile_pool(name="avp", bufs=2, space="PSUM") as avp,
            tc.tile_pool(name="sxp", bufs=6) as sxp,
            tc.tile_pool(name="outp", bufs=3) as outp,
        ):
            # per-column-pair SBUF resident operands (one [128,512] strip
            # carries slot i on partitions 0:64 and slot 9+i on 64:128)
            qcol = []
            kcol = []
            for i in range(G0):
                qt = data.tile([128, BLK], bf16, tag=f"q{i}")
                nc.sync.dma_start(out=qt, in_=qp[:, i * BLK:(i + 1) * BLK])
                qcol.append(qt)
                kt_ = data.tile([128, BLK], bf16, tag=f"k{i}")
                nc.sync.dma_start(out=kt_, in_=kp[:, i * BLK:(i + 1) * BLK])
                kcol.append(kt_)
            vcol = []
            for s in range(NSLOT):
                vt = data.tile([128, NKT * 65], bf16, tag=f"v{s}")
                nc.sync.dma_start(
                    out=vt, in_=vp[:, s * NKT * 65:(s + 1) * NKT * 65]
                )
                vcol.append(vt)

            for i in range(G0):
                slots = [i] + ([9 + i] if 9 + i < NSLOT else [])
                sxs = {s: [] for s in slots}
                for half in range(2):
                    sts = {}
                    for s in slots:
                        sts[s] = stp.tile([128, 2 * BLK], f32, tag="st", name=f"st{s}")
                    for ktl in range(2):
                        kt = half * 2 + ktl
                        for s in slots:
                            p0 = 0 if s < G0 else 64
                            nc.tensor.matmul(
                                sts[s][:, ktl * BLK:(ktl + 1) * BLK],
                                kcol[i][p0:p0 + 64, kt * KSUB:(kt + 1) * KSUB],
                                qcol[i][p0:p0 + 64, :],
                                start=True,
                                stop=True,
                                tile_position=(p0, 0),
                            )
                    for s in slots:
                        sx = sxp.tile([128, 2 * BLK], bf16, tag="sx")
                        nc.scalar.activation(
                            sx, sts[s], mybir.ActivationFunctionType.Exp,
                            scale=1.0 / DK,
                        )
                        if s < 2:  # diagonal slot: zero keys > row
                            for ktl in range(2):
                                kt = half * 2 + ktl
                                half_ap = sx[:, ktl * BLK:(ktl + 1) * BLK]
                                nc.gpsimd.affine_select(
                                    out=half_ap,
                                    in_=half_ap,
                                    pattern=[[1, BLK]],
                                    compare_op=mybir.AluOpType.is_ge,
                                    fill=0.0,
                                    base=-KSUB * kt,
                                    channel_multiplier=-1,
                                )
                        sxs[s].append(sx)
                for s in slots:
                    av = avp.tile([65, BLK], f32, tag="av")
                    for kt in range(NKT):
                        nc.tensor.matmul(
                            av,
                            vcol[s][:, kt * 65:(kt + 1) * 65],
                            sxs[s][kt // 2][:, (kt % 2) * BLK:(kt % 2 + 1) * BLK],
                            start=(kt == 0),
                            stop=(kt == NKT - 1),
                        )
                    ot = outp.tile([65, BLK], f32, tag="ot")
                    nc.vector.tensor_copy(ot, av)
                    nc.sync.dma_start(out=op[s], in_=ot)

    nc.finalize()
    return nc


def _pack_core(q_bf, k_bf, v_bf, b):
    """Build the three packed operand arrays for core b."""
    qp = np.zeros((128, QP_COLS), dtype=_BF16)
    kp = np.zeros((128, QP_COLS), dtype=_BF16)
    vp = np.zeros((128, VP_COLS), dtype=_BF16)
    slots = _core_slots(b)
    for s, (rb, cb, _diag) in enumerate(slots):
        g, i = (0, s) if s < G0 else (1, s - G0)
        p0 = 64 * g
        qp[p0:p0 + 64, i * BLK:(i + 1) * BLK] = q_bf[rb * BLK:(rb + 1) * BLK].T
        kp[p0:p0 + 64, i * BLK:(i + 1) * BLK] = k_bf[cb * BLK:(cb + 1) * BLK].T
        for kt in range(NKT):
            c0 = (s * NKT + kt) * 65
            vp[:, c0:c0 + 64] = v_bf[cb * BLK + kt * KSUB: cb * BLK + (kt + 1) * KSUB]
            vp[:, c0 + 64] = np.asarray(1.0, dtype=_BF16)
    return {"qp": qp, "kp": kp, "vp": vp}


def _combine(partials):
    """partials: list of 8 arrays [17, 65, 512] f32 -> full [8192, 64] f32."""
    out = np.empty((S, DK), dtype=np.float32)
    for b in range(N_CORES):
        slots = _core_slots(b)
        for rb in (b, 15 - b):
            idx = [s for s, (r, _c, _d) in enumerate(slots) if r == rb]
            tot = partials[b][idx].sum(axis=0)  # [65, 512]
            out[rb * BLK:(rb + 1) * BLK] = (tot[:DK] / tot[DK]).T
    return out


def kernel(q, k, v):
    from concourse.bass_utils import run_bass_kernel_spmd

    q = np.asarray(q, dtype=np.float32)
    k = np.asarray(k, dtype=np.float32)
    v = np.asarray(v, dtype=np.float32)

    if "nc" not in _CACHE:
        _CACHE["nc"] = _build_graph()
    nc = _CACHE["nc"]

    q_bf = q.astype(_BF16)
    k_bf = k.astype(_BF16)
    v_bf = v.astype(_BF16)
    in_maps = [_pack_core(q_bf, k_bf, v_bf, b) for b in range(N_CORES)]

    res = run_bass_kernel_spmd(nc, in_maps, core_ids=list(range(N_CORES)))
    partials = [np.asarray(res.results[b]["op"], dtype=np.float32)
                for b in range(N_CORES)]
    return _combine(partials)
